# revision 48
# baseline (speedup 1.0000x reference)
"""DCRNN (nn_DCRNNModel) forward pass on 8 Trainium2 NeuronCores.

Strategy: data-parallel over batch (B=32 -> 4 chains/core). All weights and
both random-walk supports are replicated per core and live in SBUF for the
whole kernel. The DCGRU cell is restructured into the diffusion "power basis"
    gconv(z) = z*W0' + (S0 z)W1 + (S0^2 z)(2 W2) + (S1 z)W3 + (S1^2 z)(2 W4)
with W0' = W0 - W2 - W4, so each projection is a plain K<=128 matmul.

Layouts (per chain):
  feature-major tiles [feat, node]: h-features on partitions 0:64,
    x-features on partitions 64:64+I (everything elementwise runs at base 0).
  node-major tiles zn [128, 8, 128]: partition=node%128, kt=node//128,
    free cols 0:64 = h-part, 64:64+I = x-part (zero padded).
Diffusion computes V1T = (S z)^T feature-major via lhsT=zn tiles, rhs=S^T
tiles; V1 is re-transposed to node-major with an identity matmul to build
V2T = (S V1)^T. Projections consume the feature-major tiles directly.
All matmuls run in float32r (full-rate fp32 on the PE).
"""
import sys
import os
import time as _time
import numpy as np

sys.path.insert(0, '/opt/trn_rl_repo')

N = 1024
B = 32
T_FULL = 12
HOR_FULL = 12
HID = 64
N_CORES = 8
CHAINS = 4           # batch elements per core
KT = 8               # node k-tiles (1024/128)
P = 128

_cache = {}


def _build(T, HOR):
    import concourse.bacc as bacc
    import concourse.tile as tile
    from concourse import mybir

    F32R = mybir.dt.float32r
    F32 = mybir.dt.float32
    BF16 = mybir.dt.bfloat16
    AF = mybir.ActivationFunctionType

    _t0 = _time.time()
    nc = bacc.Bacc('TRN2', target_bir_lowering=False, debug=False,
                   num_devices=N_CORES)

    LAYERS = (('enc0', 2), ('enc1', 64), ('dec0', 1), ('dec1', 64))

    # ---- DRAM declarations ----
    d = {}
    d['s0t'] = nc.dram_tensor('s0t', [P, KT, N], BF16, kind='ExternalInput').ap()
    d['s1t'] = nc.dram_tensor('s1t', [P, KT, N], BF16, kind='ExternalInput').ap()
    d['ident'] = nc.dram_tensor('ident', [P, P], BF16, kind='ExternalInput').ap()
    d['identf'] = nc.dram_tensor('identf', [P, P], F32R, kind='ExternalInput').ap()
    d['idhi'] = nc.dram_tensor('idhi', [P, HID], F32R, kind='ExternalInput').ap()
    d['xT'] = nc.dram_tensor('xT', [CHAINS, T, 2, N], F32R, kind='ExternalInput').ap()
    d['xn'] = nc.dram_tensor('xn', [CHAINS, T, P, KT, 2], BF16, kind='ExternalInput').ap()
    for L, I in LAYERS:
        d[L + '_wa'] = nc.dram_tensor(L + '_wa', [P, 5, P], BF16, kind='ExternalInput').ap()
        d[L + '_wc'] = nc.dram_tensor(L + '_wc', [P, 5, HID], BF16, kind='ExternalInput').ap()
        d[L + '_wa0'] = nc.dram_tensor(L + '_wa0', [P, P], F32R, kind='ExternalInput').ap()
        d[L + '_wc0h'] = nc.dram_tensor(L + '_wc0h', [HID, HID], BF16, kind='ExternalInput').ap()
        d[L + '_bru'] = nc.dram_tensor(L + '_bru', [P, 1], F32, kind='ExternalInput').ap()
        d[L + '_bc'] = nc.dram_tensor(L + '_bc', [HID, 1], F32, kind='ExternalInput').ap()
        if I == 64:
            d[L + '_wa0x'] = nc.dram_tensor(L + '_wa0x', [HID, P], F32R, kind='ExternalInput').ap()
            d[L + '_wc0x'] = nc.dram_tensor(L + '_wc0x', [HID, HID], F32R, kind='ExternalInput').ap()
    d['enc0_wc0'] = nc.dram_tensor('enc0_wc0', [P, HID], F32R, kind='ExternalInput').ap()
    d['dec0_wa0x1'] = nc.dram_tensor('dec0_wa0x1', [P, P], BF16, kind='ExternalInput').ap()
    d['dec0_wc0x1'] = nc.dram_tensor('dec0_wc0x1', [P, HID], BF16, kind='ExternalInput').ap()
    d['fcnw'] = nc.dram_tensor('fcnw', [HID, 1], F32R, kind='ExternalInput').ap()
    d['fcnw8'] = nc.dram_tensor('fcnw8', [HID, 8], F32R, kind='ExternalInput').ap()
    d['fcnb'] = nc.dram_tensor('fcnb', [P, 1], F32, kind='ExternalInput').ap()
    d['zeros'] = nc.dram_tensor('zeros', [P, N], F32R, kind='ExternalInput').ap()
    d['zerosb'] = nc.dram_tensor('zerosb', [P, N], BF16, kind='ExternalInput').ap()
    d_out = nc.dram_tensor('out', [HOR, CHAINS, N], F32, kind='ExternalOutput').ap()

    _temit = _time.time()
    with tile.TileContext(nc) as tc:
        with tc.tile_pool(name='const', bufs=1) as const, \
             tc.tile_pool(name='state', bufs=1) as state, \
             tc.tile_pool(name='vt', bufs=6) as vtp, \
             tc.tile_pool(name='vn', bufs=2) as vnp, \
             tc.tile_pool(name='ru', bufs=2) as rup, \
             tc.tile_pool(name='u0', bufs=2) as u0p, \
             tc.tile_pool(name='rh', bufs=2) as rhp, \
             tc.tile_pool(name='ct', bufs=2) as ctp, \
             tc.tile_pool(name='tg', bufs=2) as tgp, \
             tc.tile_pool(name='yt', bufs=1) as ytp, \
             tc.tile_pool(name='pd', bufs=4, space='PSUM') as pdp, \
             tc.tile_pool(name='pp', bufs=2, space='PSUM') as ppp, \
             tc.tile_pool(name='pt', bufs=2, space='PSUM') as ptp:

            # ---- load constants ----
            cst = {}
            for name, dd in d.items():
                if name in ('xT', 'xn', 'zeros', 'zerosb'):
                    continue
                shape = list(dd.shape)
                dt_ = dd.dtype
                t = const.tile(shape, dt_, tag=name, name='cst_' + name)
                nc.sync.dma_start(t[:], dd[:])
                cst[name] = t
            s0t, s1t = cst['s0t'], cst['s1t']
            ident, idhi = cst['ident'], cst['idhi']
            identf = cst['identf']

            # ---- persistent state ----
            d_zeros = d['zeros']
            d_zerosb = d['zerosb']
            zn_zeros = d_zerosb.rearrange('p (kt f) -> p kt f', kt=KT)
            zT0, zT1, zn0, zn1 = [], [], [], []
            yall = state.tile([P, N], BF16, tag='yall', name='yall')
            nc.sync.dma_start(yall[:], d_zerosb[:])
            for c in range(CHAINS):
                zT0.append(state.tile([P, N], F32R, tag=f'zT0_{c}', name=f'zT0_{c}'))
                zT1.append(state.tile([HID, N], F32R, tag=f'zT1_{c}', name=f'zT1_{c}'))
                zn0.append(state.tile([P, KT, P], BF16, tag=f'zn0_{c}', name=f'zn0_{c}'))
                zn1.append(state.tile([P, KT, P], BF16, tag=f'zn1_{c}', name=f'zn1_{c}'))
                nc.sync.dma_start(zT0[c][:], d_zeros[:])
                nc.sync.dma_start(zT1[c][:], d_zeros[0:HID, :])
                nc.sync.dma_start(zn0[c][:], zn_zeros[:])
                nc.sync.dma_start(zn1[c][:], zn_zeros[:])

            def diffuse(zn_tile, stt):
                """V = (S z)^T feature-major [128, N] (bf16 sbuf tile)."""
                v = vtp.tile([P, N], BF16, tag='vt')
                for ms in range(2):
                    sl = slice(ms * 512, (ms + 1) * 512)
                    p = pdp.tile([P, 512], F32, tag='pd')
                    for kt in range(KT):
                        nc.tensor.matmul(p[:], zn_tile[:, kt, :],
                                         stt[:, kt, sl],
                                         start=(kt == 0), stop=(kt == KT - 1))
                    if ms == 0:
                        nc.scalar.copy(v[:, sl], p[:])
                    else:
                        nc.vector.tensor_copy(v[:, sl], p[:])
                return v

            def to_node(vt_tile):
                """feature-major [128, N] -> node-major [128, KT, 128]."""
                v = vnp.tile([P, KT, P], BF16, tag='vn')
                for half in range(2):
                    p = ptp.tile([P, KT // 2, P], F32, tag='pt')
                    for m in range(KT // 2):
                        mm = half * (KT // 2) + m
                        nc.tensor.matmul(p[:, m, :], vt_tile[:, mm * P:(mm + 1) * P],
                                         ident[:], start=True, stop=True)
                    if half == 0:
                        nc.scalar.copy(v[:, 0:KT // 2, :], p[:])
                    else:
                        nc.vector.tensor_copy(v[:, KT // 2:KT, :], p[:])
                return v

            def gconv_vtiles(zn_tile):
                vts = []
                for stt in (s0t, s1t):
                    v1 = diffuse(zn_tile, stt)
                    v1n = to_node(v1)
                    v2 = diffuse(v1n, stt)
                    vts += [v1, v2]
                return vts

            def cell(L, I, c, is_layer0, t_idx, phase):
                znL = zn0[c] if is_layer0 else zn1[c]
                zTh = zT0[c] if is_layer0 else zT1[c]   # h rows 0:64
                wa = cst[L + '_wa']
                wc = cst[L + '_wc']

                # --- gA ---
                vts = gconv_vtiles(znL)
                ru = rup.tile([P, N], F32R, tag='ru')
                for ms in range(2):
                    sl = slice(ms * 512, (ms + 1) * 512)
                    pa = ppp.tile([P, 512], F32, tag='pp')
                    mms = []
                    # identity terms
                    if L == 'enc0':
                        mms.append((cst[L + '_wa0'][:], zT0[c][:, sl], None))
                    elif L == 'dec0':
                        mms.append((cst[L + '_wa0'][:], zT0[c][:, sl], None))
                        b32 = 32 * c
                        mms.append((cst['dec0_wa0x1'][b32:b32 + 1, :],
                                    yall[b32:b32 + 1, sl], (b32, 0)))
                    else:
                        mms.append((cst[L + '_wa0'][0:HID, :], zTh[0:HID, sl], None))
                        mms.append((cst[L + '_wa0x'][:], zT0[c][0:HID, sl], None))
                    for i in range(4):
                        mms.append((wa[:, i + 1, :], vts[i][:, sl], None))
                    for j, (lhs, rhs, tp) in enumerate(mms):
                        nc.tensor.matmul(pa[:], lhs, rhs,
                                         start=(j == 0), stop=(j == len(mms) - 1),
                                         tile_position=tp)
                    nc.scalar.activation(ru[:, sl], pa[:], AF.Sigmoid,
                                         bias=cst[L + '_bru'][:])
                # u = ru rows 64:128 shifted down to base 0 via idhi
                u0 = u0p.tile([HID, N], F32, tag='u0')
                for ms in range(2):
                    sl = slice(ms * 512, (ms + 1) * 512)
                    psh = ptp.tile([P, 512], F32, tag='pt')
                    nc.tensor.matmul(psh[0:HID, :], idhi[:], ru[:, sl],
                                     start=True, stop=True)
                    if ms == 0:
                        nc.scalar.copy(u0[:, sl], psh[0:HID, :])
                    else:
                        nc.vector.tensor_copy(u0[:, sl], psh[0:HID, :])
                # rh = r * h
                rh = rhp.tile([HID, N], BF16, tag='rh')
                nc.vector.tensor_mul(rh[:], ru[0:HID, :], zTh[0:HID, :])
                # rh -> node (overwrite h-slot of znL)
                H2 = KT // 2
                for half in range(2):
                    prh = ptp.tile([P, H2, HID], F32, tag='pt')
                    for m in range(H2):
                        mm = half * H2 + m
                        nc.tensor.matmul(prh[:, m, :], rh[:, mm * P:(mm + 1) * P],
                                         ident[0:HID, 0:HID], start=True, stop=True)
                    if half == 0:
                        nc.scalar.copy(znL[:, 0:H2, 0:HID], prh[:])
                    else:
                        nc.vector.tensor_copy(znL[:, H2:KT, 0:HID], prh[:])

                # --- gB ---
                vts2 = gconv_vtiles(znL)
                ct = ctp.tile([HID, N], F32, tag='ct')
                for ms in range(2):
                    sl = slice(ms * 512, (ms + 1) * 512)
                    pb = ppp.tile([P, 512], F32, tag='pp')
                    mms = [(cst[L + '_wc0h'][:], rh[:, sl], None)]
                    if L == 'enc0':
                        mms.append((cst['enc0_wc0'][:], zT0[c][:, sl], None))
                    elif L == 'dec0':
                        b32 = 32 * c
                        mms.append((cst['dec0_wc0x1'][b32:b32 + 1, :],
                                    yall[b32:b32 + 1, sl], (b32, 0)))
                    else:
                        mms.append((cst[L + '_wc0x'][:], zT0[c][0:HID, sl], None))
                    for i in range(4):
                        mms.append((wc[:, i + 1, :], vts2[i][:, sl], None))
                    for j, (lhs, rhs, tp) in enumerate(mms):
                        nc.tensor.matmul(pb[0:HID], lhs, rhs,
                                         start=(j == 0), stop=(j == len(mms) - 1),
                                         tile_position=tp)
                    nc.scalar.activation(ct[:, sl], pb[0:HID], AF.Tanh,
                                         bias=cst[L + '_bc'][:])
                # --- GRU: h' = c + u*(h-c) ---
                t1 = tgp.tile([HID, N], F32, tag='tg')
                nc.vector.tensor_sub(t1[:], zTh[0:HID, :], ct[:])
                nc.vector.tensor_mul(t1[:], t1[:], u0[:])
                nc.vector.tensor_add(zTh[0:HID, :], ct[:], t1[:])
                # --- h' -> node ---
                H2 = KT // 2
                for half in range(2):
                    ph = ptp.tile([P, H2, HID], F32, tag='pt')
                    for m in range(H2):
                        mm = half * H2 + m
                        nc.tensor.matmul(ph[:, m, :], zTh[0:HID, mm * P:(mm + 1) * P],
                                         identf[0:HID, 0:HID], start=True, stop=True)
                    msl = slice(half * H2, half * H2 + H2)
                    if half == 0:
                        nc.scalar.copy(znL[:, msl, 0:HID], ph[:])
                        if is_layer0:
                            nc.vector.tensor_copy(zn1[c][:, msl, HID:P], ph[:])
                    else:
                        nc.vector.tensor_copy(znL[:, msl, 0:HID], ph[:])
                        if is_layer0:
                            nc.scalar.copy(zn1[c][:, msl, HID:P], ph[:])

            # ================= encoder =================
            for t in range(T):
                for c in range(CHAINS):
                    nc.sync.dma_start(zT0[c][HID:HID + 2, :], d['xT'][c, t])
                    nc.sync.dma_start(zn0[c][:, :, HID:HID + 2], d['xn'][c, t])
                for c in range(CHAINS):
                    cell('enc0', 2, c, True, t, 'enc')
                for c in range(CHAINS):
                    cell('enc1', 64, c, False, t, 'enc')

            # ================= decoder =================
            for c in range(CHAINS):
                nc.sync.dma_start(zT0[c][HID:HID + 2, :], d_zeros[0:2, :])
                nc.sync.dma_start(zn0[c][:, :, HID:HID + 2],
                                  d_zerosb[:, 0:2 * KT].rearrange('p (kt f) -> p kt f', kt=KT))
            nc.sync.dma_start(yall[:], d_zerosb[:])
            for t in range(HOR):
                for c in range(CHAINS):
                    cell('dec0', 1, c, True, t, 'dec')
                for c in range(CHAINS):
                    cell('dec1', 64, c, False, t, 'dec')
                    # fcn: yT and y-node from h1' (zT1 rows 0:64)
                    ytmp = ytp.tile([1, N], F32, tag='yt', name='ytmp')
                    for ms in range(2):
                        sl = slice(ms * 512, (ms + 1) * 512)
                        py = ptp.tile([P, 512], F32, tag='pt')
                        nc.tensor.matmul(py[0:1, :], cst['fcnw'][:],
                                         zT1[c][0:HID, sl], start=True, stop=True)
                        nc.scalar.activation(ytmp[:, sl], py[0:1, :], AF.Identity,
                                             bias=cst['fcnb'][0:1, :])
                        nc.scalar.activation(yall[32 * c:32 * c + 1, sl], py[0:1, :],
                                             AF.Identity, bias=cst['fcnb'][0:1, :])
                    nc.sync.dma_start(d_out[t, c:c + 1, :], ytmp[:])
                    H2 = KT // 2
                    for half in range(2):
                        pyn = ptp.tile([P, H2, 8], F32, tag='pt')
                        for m in range(H2):
                            mm = half * H2 + m
                            nc.tensor.matmul(pyn[:, m, :],
                                             zT1[c][0:HID, mm * P:(mm + 1) * P],
                                             cst['fcnw8'][:], start=True, stop=True)
                        msl = slice(half * H2, half * H2 + H2)
                        nc.scalar.activation(zn0[c][:, msl, HID:HID + 1],
                                             pyn[:, :, 0:1], AF.Identity,
                                             bias=cst['fcnb'][:])

    print(f'[build] emission+schedule: {_time.time() - _t0:.1f}s', flush=True)
    _t1 = _time.time()
    nc.compile()
    print(f'[build] bacc compile: {_time.time() - _t1:.1f}s', flush=True)
    return nc


def _prep_host(inputs):
    """Host-side preprocessing -> per-core in_maps."""
    import ml_dtypes
    f32 = np.float32
    bf = ml_dtypes.bfloat16
    adj = np.asarray(inputs['adj'], f32)
    source = np.asarray(inputs['source'], f32)       # [B, T, N, 2]

    def rw(a):
        return (a / np.maximum(a.sum(1, keepdims=True), np.float32(1e-8))).astype(f32)

    s0t = rw(adj)            # rhs tiles: ST[n, m] with S0 = rw(adj).T
    s1t = rw(adj.T)

    def tile_nm(a):          # [N, N] -> [128, KT, N]
        return np.ascontiguousarray(a.reshape(KT, P, N).transpose(1, 0, 2))

    common = {
        's0t': tile_nm(s0t).astype(bf),
        's1t': tile_nm(s1t).astype(bf),
        'ident': np.eye(P, dtype=bf),
        'identf': np.eye(P, dtype=f32),
    }
    idhi = np.zeros((P, HID), f32)
    idhi[HID:P] = np.eye(HID, dtype=f32)
    common['idhi'] = idhi

    for L, I in (('enc0', 2), ('enc1', 64), ('dec0', 1), ('dec1', 64)):
        F = I + HID
        Wru = np.asarray(inputs[L + '_Wru'], f32)
        Wc = np.asarray(inputs[L + '_Wc'], f32)
        bru = np.asarray(inputs[L + '_bru'], f32)
        bc = np.asarray(inputs[L + '_bc'], f32)
        bA = [Wru[i * F:(i + 1) * F] for i in range(5)]
        bC = [Wc[i * F:(i + 1) * F] for i in range(5)]
        eA = [bA[0] - bA[2] - bA[4], bA[1], 2 * bA[2], bA[3], 2 * bA[4]]
        eC = [bC[0] - bC[2] - bC[4], bC[1], 2 * bC[2], bC[3], 2 * bC[4]]
        wa = np.zeros((P, 5, P), f32)
        wc = np.zeros((P, 5, HID), f32)
        for i in range(5):
            wa[0:HID, i] = eA[i][I:F]        # h-part rows 0:64
            wa[HID:HID + I, i] = eA[i][0:I]  # x-part rows 64:64+I
            if i > 0:
                wc[0:HID, i] = eC[i][I:F]
                wc[HID:HID + I, i] = eC[i][0:I]
        # gB identity: h-part via rh tile (wc0h), x-part per layer
        common[L + '_wc0h'] = np.ascontiguousarray(eC[0][I:F]).astype(bf)
        if L == 'enc0':
            wc0 = np.zeros((P, HID), f32)
            wc0[HID:HID + I] = eC[0][0:I]    # x-part read from zT0 (K=128)
            common['enc0_wc0'] = wc0
        elif L == 'dec0':
            wa0x1 = np.zeros((P, P), f32)
            wc0x1 = np.zeros((P, HID), f32)
            for cc in range(4):
                wa0x1[32 * cc] = eA[0][0]
                wc0x1[32 * cc] = eC[0][0]
            common['dec0_wa0x1'] = wa0x1.astype(bf)
            common['dec0_wc0x1'] = wc0x1.astype(bf)
            wa[HID:HID + I, 0] = 0           # y handled via K=1 terms
        else:
            common[L + '_wa0x'] = np.ascontiguousarray(eA[0][0:I])
            common[L + '_wc0x'] = np.ascontiguousarray(eC[0][0:I])
        common[L + '_wa0'] = np.ascontiguousarray(wa[:, 0, :])
        common[L + '_wa'] = wa.astype(bf)
        common[L + '_wc'] = wc.astype(bf)
        common[L + '_bru'] = bru.reshape(P, 1).copy()
        common[L + '_bc'] = bc.reshape(HID, 1).copy()

    common['zeros'] = np.zeros((P, N), f32)
    common['zerosb'] = np.zeros((P, N), bf)
    common['fcnw'] = np.asarray(inputs['fcn_W'], f32).reshape(HID, 1).copy()
    common['fcnw8'] = np.repeat(common['fcnw'], 8, axis=1).copy()
    common['fcnb'] = np.full((P, 1), np.asarray(inputs['fcn_b'], f32).reshape(-1)[0], f32)

    T = _cache['T']
    in_maps = []
    for core in range(N_CORES):
        m = dict(common)
        xT = np.zeros((CHAINS, T, 2, N), f32)
        xn = np.zeros((CHAINS, T, P, KT, 2), bf)
        for c in range(CHAINS):
            b = core * CHAINS + c
            for t in range(T):
                xt = source[b, t]            # [N, 2]
                xT[c, t] = xt.T
                xn[c, t] = xt.reshape(KT, P, 2).transpose(1, 0, 2)
        m['xT'] = xT
        m['xn'] = xn
        in_maps.append(m)
    return in_maps


def kernel(**inputs):
    from concourse import bass_utils

    T = int(os.environ.get('DCRNN_T', T_FULL))
    HOR = int(os.environ.get('DCRNN_HOR', HOR_FULL))
    key = (T, HOR)
    if _cache.get('key') != key:
        _cache['nc'] = _build(T, HOR)
        _cache['key'] = key
        _cache['T'] = T
        _cache['HOR'] = HOR
    _cache['T'] = T

    in_maps = _prep_host(inputs)
    res = bass_utils.run_bass_kernel_spmd(
        _cache['nc'], in_maps, core_ids=list(range(N_CORES)),
        trace=bool(int(os.environ.get('DCRNN_TRACE', '0'))))
    _cache['last_res'] = res

    HORr = _cache['HOR']
    out = np.zeros((HORr, B, N), np.float32)
    for core in range(N_CORES):
        r = res.results[core]['out']         # [HOR, CHAINS, N]
        for c in range(CHAINS):
            out[:, core * CHAINS + c, :] = r[:, c, :]
    return out



# revision 51
# speedup vs baseline: 1.0022x; 1.0022x over previous
"""DCRNN (nn_DCRNNModel) forward pass on 8 Trainium2 NeuronCores.

Strategy: data-parallel over batch (B=32 -> 4 chains/core). All weights and
both random-walk supports are replicated per core and live in SBUF for the
whole kernel. The DCGRU cell is restructured into the diffusion "power basis"
    gconv(z) = z*W0' + (S0 z)W1 + (S0^2 z)(2 W2) + (S1 z)W3 + (S1^2 z)(2 W4)
with W0' = W0 - W2 - W4, so each projection is a plain K<=128 matmul.

Layouts (per chain):
  feature-major tiles [feat, node]: h-features on partitions 0:64,
    x-features on partitions 64:64+I (everything elementwise runs at base 0).
  node-major tiles zn [128, 8, 128]: partition=node%128, kt=node//128,
    free cols 0:64 = h-part, 64:64+I = x-part (zero padded).
Diffusion computes V1T = (S z)^T feature-major via lhsT=zn tiles, rhs=S^T
tiles; V1 is re-transposed to node-major with an identity matmul to build
V2T = (S V1)^T. Projections consume the feature-major tiles directly.
All matmuls run in float32r (full-rate fp32 on the PE).
"""
import sys
import os
import time as _time
import numpy as np

sys.path.insert(0, '/opt/trn_rl_repo')

N = 1024
B = 32
T_FULL = 12
HOR_FULL = 12
HID = 64
N_CORES = 8
CHAINS = 4           # batch elements per core
KT = 8               # node k-tiles (1024/128)
P = 128

_cache = {}


def _build(T, HOR):
    import concourse.bacc as bacc
    import concourse.tile as tile
    from concourse import mybir

    F32R = mybir.dt.float32r
    F32 = mybir.dt.float32
    BF16 = mybir.dt.bfloat16
    AF = mybir.ActivationFunctionType

    _t0 = _time.time()
    nc = bacc.Bacc('TRN2', target_bir_lowering=False, debug=False,
                   num_devices=N_CORES)

    LAYERS = (('enc0', 2), ('enc1', 64), ('dec0', 1), ('dec1', 64))

    # ---- DRAM declarations ----
    d = {}
    d['s0t'] = nc.dram_tensor('s0t', [P, KT, N], BF16, kind='ExternalInput').ap()
    d['s1t'] = nc.dram_tensor('s1t', [P, KT, N], BF16, kind='ExternalInput').ap()
    d['ident'] = nc.dram_tensor('ident', [P, P], BF16, kind='ExternalInput').ap()
    d['identf'] = nc.dram_tensor('identf', [P, P], F32R, kind='ExternalInput').ap()
    d['idhi'] = nc.dram_tensor('idhi', [P, HID], F32R, kind='ExternalInput').ap()
    d['xT'] = nc.dram_tensor('xT', [CHAINS, T, 2, N], F32R, kind='ExternalInput').ap()
    d['xn'] = nc.dram_tensor('xn', [CHAINS, T, P, KT, 2], BF16, kind='ExternalInput').ap()
    for L, I in LAYERS:
        d[L + '_wa'] = nc.dram_tensor(L + '_wa', [P, 5, P], BF16, kind='ExternalInput').ap()
        d[L + '_wc'] = nc.dram_tensor(L + '_wc', [P, 5, HID], BF16, kind='ExternalInput').ap()
        d[L + '_wa0'] = nc.dram_tensor(L + '_wa0', [P, P], F32R, kind='ExternalInput').ap()
        d[L + '_wc0h'] = nc.dram_tensor(L + '_wc0h', [HID, HID], BF16, kind='ExternalInput').ap()
        d[L + '_bru'] = nc.dram_tensor(L + '_bru', [P, 1], F32, kind='ExternalInput').ap()
        d[L + '_bc'] = nc.dram_tensor(L + '_bc', [HID, 1], F32, kind='ExternalInput').ap()
        if I == 64:
            d[L + '_wa0x'] = nc.dram_tensor(L + '_wa0x', [HID, P], F32R, kind='ExternalInput').ap()
            d[L + '_wc0x'] = nc.dram_tensor(L + '_wc0x', [HID, HID], F32R, kind='ExternalInput').ap()
    d['enc0_wc0'] = nc.dram_tensor('enc0_wc0', [P, HID], F32R, kind='ExternalInput').ap()
    d['dec0_wa0x1'] = nc.dram_tensor('dec0_wa0x1', [P, P], BF16, kind='ExternalInput').ap()
    d['dec0_wc0x1'] = nc.dram_tensor('dec0_wc0x1', [P, HID], BF16, kind='ExternalInput').ap()
    d['fcnw'] = nc.dram_tensor('fcnw', [HID, 1], F32R, kind='ExternalInput').ap()
    d['fcnw8'] = nc.dram_tensor('fcnw8', [HID, 8], F32R, kind='ExternalInput').ap()
    d['fcnb'] = nc.dram_tensor('fcnb', [P, 1], F32, kind='ExternalInput').ap()
    d['zeros'] = nc.dram_tensor('zeros', [P, N], F32R, kind='ExternalInput').ap()
    d['zerosb'] = nc.dram_tensor('zerosb', [P, N], BF16, kind='ExternalInput').ap()
    d_out = nc.dram_tensor('out', [HOR, CHAINS, N], F32, kind='ExternalOutput').ap()

    _temit = _time.time()
    with tile.TileContext(nc) as tc:
        with tc.tile_pool(name='const', bufs=1) as const, \
             tc.tile_pool(name='state', bufs=1) as state, \
             tc.tile_pool(name='vt', bufs=8) as vtp, \
             tc.tile_pool(name='vn', bufs=2) as vnp, \
             tc.tile_pool(name='ru', bufs=2) as rup, \
             tc.tile_pool(name='u0', bufs=2) as u0p, \
             tc.tile_pool(name='rh', bufs=2) as rhp, \
             tc.tile_pool(name='ct', bufs=2) as ctp, \
             tc.tile_pool(name='tg', bufs=2) as tgp, \
             tc.tile_pool(name='yt', bufs=1) as ytp, \
             tc.tile_pool(name='pd', bufs=4, space='PSUM') as pdp, \
             tc.tile_pool(name='pp', bufs=2, space='PSUM') as ppp, \
             tc.tile_pool(name='pt', bufs=2, space='PSUM') as ptp:

            # ---- load constants ----
            cst = {}
            for name, dd in d.items():
                if name in ('xT', 'xn', 'zeros', 'zerosb'):
                    continue
                shape = list(dd.shape)
                dt_ = dd.dtype
                t = const.tile(shape, dt_, tag=name, name='cst_' + name)
                nc.sync.dma_start(t[:], dd[:])
                cst[name] = t
            s0t, s1t = cst['s0t'], cst['s1t']
            ident, idhi = cst['ident'], cst['idhi']
            identf = cst['identf']

            # ---- persistent state ----
            d_zeros = d['zeros']
            d_zerosb = d['zerosb']
            zn_zeros = d_zerosb.rearrange('p (kt f) -> p kt f', kt=KT)
            zT0, zT1, zn0, zn1 = [], [], [], []
            yall = state.tile([P, N], BF16, tag='yall', name='yall')
            nc.sync.dma_start(yall[:], d_zerosb[:])
            for c in range(CHAINS):
                zT0.append(state.tile([P, N], F32R, tag=f'zT0_{c}', name=f'zT0_{c}'))
                zT1.append(state.tile([HID, N], F32R, tag=f'zT1_{c}', name=f'zT1_{c}'))
                zn0.append(state.tile([P, KT, P], BF16, tag=f'zn0_{c}', name=f'zn0_{c}'))
                zn1.append(state.tile([P, KT, P], BF16, tag=f'zn1_{c}', name=f'zn1_{c}'))
                nc.sync.dma_start(zT0[c][:], d_zeros[:])
                nc.sync.dma_start(zT1[c][:], d_zeros[0:HID, :])
                nc.sync.dma_start(zn0[c][:], zn_zeros[:])
                nc.sync.dma_start(zn1[c][:], zn_zeros[:])

            def diffuse(zn_tile, stt):
                """V = (S z)^T feature-major [128, N] (bf16 sbuf tile)."""
                v = vtp.tile([P, N], BF16, tag='vt')
                for ms in range(2):
                    sl = slice(ms * 512, (ms + 1) * 512)
                    p = pdp.tile([P, 512], F32, tag='pd')
                    for kt in range(KT):
                        nc.tensor.matmul(p[:], zn_tile[:, kt, :],
                                         stt[:, kt, sl],
                                         start=(kt == 0), stop=(kt == KT - 1))
                    if ms == 0:
                        nc.scalar.copy(v[:, sl], p[:])
                    else:
                        nc.vector.tensor_copy(v[:, sl], p[:])
                return v

            def to_node(vt_tile):
                """feature-major [128, N] -> node-major [128, KT, 128]."""
                v = vnp.tile([P, KT, P], BF16, tag='vn')
                for half in range(2):
                    p = ptp.tile([P, KT // 2, P], F32, tag='pt')
                    for m in range(KT // 2):
                        mm = half * (KT // 2) + m
                        nc.tensor.matmul(p[:, m, :], vt_tile[:, mm * P:(mm + 1) * P],
                                         ident[:], start=True, stop=True)
                    if half == 0:
                        nc.scalar.copy(v[:, 0:KT // 2, :], p[:])
                    else:
                        nc.vector.tensor_copy(v[:, KT // 2:KT, :], p[:])
                return v

            def gconv_vtiles(zn_tile):
                vts = []
                for stt in (s0t, s1t):
                    v1 = diffuse(zn_tile, stt)
                    v1n = to_node(v1)
                    v2 = diffuse(v1n, stt)
                    vts += [v1, v2]
                return vts

            def cell(L, I, c, is_layer0, t_idx, phase):
                znL = zn0[c] if is_layer0 else zn1[c]
                zTh = zT0[c] if is_layer0 else zT1[c]   # h rows 0:64
                wa = cst[L + '_wa']
                wc = cst[L + '_wc']

                # --- gA ---
                vts = gconv_vtiles(znL)
                ru = rup.tile([P, N], F32R, tag='ru')
                for ms in range(2):
                    sl = slice(ms * 512, (ms + 1) * 512)
                    pa = ppp.tile([P, 512], F32, tag='pp')
                    mms = []
                    # identity terms
                    if L == 'enc0':
                        mms.append((cst[L + '_wa0'][:], zT0[c][:, sl], None))
                    elif L == 'dec0':
                        mms.append((cst[L + '_wa0'][:], zT0[c][:, sl], None))
                        b32 = 32 * c
                        mms.append((cst['dec0_wa0x1'][b32:b32 + 1, :],
                                    yall[b32:b32 + 1, sl], (b32, 0)))
                    else:
                        mms.append((cst[L + '_wa0'][0:HID, :], zTh[0:HID, sl], None))
                        mms.append((cst[L + '_wa0x'][:], zT0[c][0:HID, sl], None))
                    for i in range(4):
                        mms.append((wa[:, i + 1, :], vts[i][:, sl], None))
                    for j, (lhs, rhs, tp) in enumerate(mms):
                        nc.tensor.matmul(pa[:], lhs, rhs,
                                         start=(j == 0), stop=(j == len(mms) - 1),
                                         tile_position=tp)
                    nc.scalar.activation(ru[:, sl], pa[:], AF.Sigmoid,
                                         bias=cst[L + '_bru'][:])
                # u = ru rows 64:128 shifted down to base 0 via idhi
                u0 = u0p.tile([HID, N], F32, tag='u0')
                for ms in range(2):
                    sl = slice(ms * 512, (ms + 1) * 512)
                    psh = ptp.tile([P, 512], F32, tag='pt')
                    nc.tensor.matmul(psh[0:HID, :], idhi[:], ru[:, sl],
                                     start=True, stop=True)
                    if ms == 0:
                        nc.scalar.copy(u0[:, sl], psh[0:HID, :])
                    else:
                        nc.vector.tensor_copy(u0[:, sl], psh[0:HID, :])
                # rh = r * h
                rh = rhp.tile([HID, N], BF16, tag='rh')
                nc.vector.tensor_mul(rh[:], ru[0:HID, :], zTh[0:HID, :])
                # rh -> node (overwrite h-slot of znL)
                H2 = KT // 2
                for half in range(2):
                    prh = ptp.tile([P, H2, HID], F32, tag='pt')
                    for m in range(H2):
                        mm = half * H2 + m
                        nc.tensor.matmul(prh[:, m, :], rh[:, mm * P:(mm + 1) * P],
                                         ident[0:HID, 0:HID], start=True, stop=True)
                    if half == 0:
                        nc.scalar.copy(znL[:, 0:H2, 0:HID], prh[:])
                    else:
                        nc.vector.tensor_copy(znL[:, H2:KT, 0:HID], prh[:])

                # --- gB ---
                vts2 = gconv_vtiles(znL)
                ct = ctp.tile([HID, N], F32, tag='ct')
                for ms in range(2):
                    sl = slice(ms * 512, (ms + 1) * 512)
                    pb = ppp.tile([P, 512], F32, tag='pp')
                    mms = [(cst[L + '_wc0h'][:], rh[:, sl], None)]
                    if L == 'enc0':
                        mms.append((cst['enc0_wc0'][:], zT0[c][:, sl], None))
                    elif L == 'dec0':
                        b32 = 32 * c
                        mms.append((cst['dec0_wc0x1'][b32:b32 + 1, :],
                                    yall[b32:b32 + 1, sl], (b32, 0)))
                    else:
                        mms.append((cst[L + '_wc0x'][:], zT0[c][0:HID, sl], None))
                    for i in range(4):
                        mms.append((wc[:, i + 1, :], vts2[i][:, sl], None))
                    for j, (lhs, rhs, tp) in enumerate(mms):
                        nc.tensor.matmul(pb[0:HID], lhs, rhs,
                                         start=(j == 0), stop=(j == len(mms) - 1),
                                         tile_position=tp)
                    nc.scalar.activation(ct[:, sl], pb[0:HID], AF.Tanh,
                                         bias=cst[L + '_bc'][:])
                # --- GRU: h' = c + u*(h-c) ---
                t1 = tgp.tile([HID, N], F32, tag='tg')
                nc.vector.tensor_sub(t1[:], zTh[0:HID, :], ct[:])
                nc.vector.tensor_mul(t1[:], t1[:], u0[:])
                nc.vector.tensor_add(zTh[0:HID, :], ct[:], t1[:])
                # --- h' -> node ---
                H2 = KT // 2
                for half in range(2):
                    ph = ptp.tile([P, H2, HID], F32, tag='pt')
                    for m in range(H2):
                        mm = half * H2 + m
                        nc.tensor.matmul(ph[:, m, :], zTh[0:HID, mm * P:(mm + 1) * P],
                                         identf[0:HID, 0:HID], start=True, stop=True)
                    msl = slice(half * H2, half * H2 + H2)
                    if half == 0:
                        nc.scalar.copy(znL[:, msl, 0:HID], ph[:])
                        if is_layer0:
                            nc.vector.tensor_copy(zn1[c][:, msl, HID:P], ph[:])
                    else:
                        nc.vector.tensor_copy(znL[:, msl, 0:HID], ph[:])
                        if is_layer0:
                            nc.scalar.copy(zn1[c][:, msl, HID:P], ph[:])

            # ================= encoder =================
            for t in range(T):
                for c in range(CHAINS):
                    nc.sync.dma_start(zT0[c][HID:HID + 2, :], d['xT'][c, t])
                    nc.sync.dma_start(zn0[c][:, :, HID:HID + 2], d['xn'][c, t])
                for c in range(CHAINS):
                    cell('enc0', 2, c, True, t, 'enc')
                for c in range(CHAINS):
                    cell('enc1', 64, c, False, t, 'enc')

            # ================= decoder =================
            for c in range(CHAINS):
                nc.sync.dma_start(zT0[c][HID:HID + 2, :], d_zeros[0:2, :])
                nc.sync.dma_start(zn0[c][:, :, HID:HID + 2],
                                  d_zerosb[:, 0:2 * KT].rearrange('p (kt f) -> p kt f', kt=KT))
            nc.sync.dma_start(yall[:], d_zerosb[:])
            for t in range(HOR):
                for c in range(CHAINS):
                    cell('dec0', 1, c, True, t, 'dec')
                for c in range(CHAINS):
                    cell('dec1', 64, c, False, t, 'dec')
                    # fcn: yT and y-node from h1' (zT1 rows 0:64)
                    ytmp = ytp.tile([1, N], F32, tag='yt', name='ytmp')
                    for ms in range(2):
                        sl = slice(ms * 512, (ms + 1) * 512)
                        py = ptp.tile([P, 512], F32, tag='pt')
                        nc.tensor.matmul(py[0:1, :], cst['fcnw'][:],
                                         zT1[c][0:HID, sl], start=True, stop=True)
                        nc.scalar.activation(ytmp[:, sl], py[0:1, :], AF.Identity,
                                             bias=cst['fcnb'][0:1, :])
                        nc.scalar.activation(yall[32 * c:32 * c + 1, sl], py[0:1, :],
                                             AF.Identity, bias=cst['fcnb'][0:1, :])
                    nc.sync.dma_start(d_out[t, c:c + 1, :], ytmp[:])
                    H2 = KT // 2
                    for half in range(2):
                        pyn = ptp.tile([P, H2, 8], F32, tag='pt')
                        for m in range(H2):
                            mm = half * H2 + m
                            nc.tensor.matmul(pyn[:, m, :],
                                             zT1[c][0:HID, mm * P:(mm + 1) * P],
                                             cst['fcnw8'][:], start=True, stop=True)
                        msl = slice(half * H2, half * H2 + H2)
                        nc.scalar.activation(zn0[c][:, msl, HID:HID + 1],
                                             pyn[:, :, 0:1], AF.Identity,
                                             bias=cst['fcnb'][:])

    print(f'[build] emission+schedule: {_time.time() - _t0:.1f}s', flush=True)
    _t1 = _time.time()
    nc.compile()
    print(f'[build] bacc compile: {_time.time() - _t1:.1f}s', flush=True)
    return nc


def _prep_host(inputs):
    """Host-side preprocessing -> per-core in_maps."""
    import ml_dtypes
    f32 = np.float32
    bf = ml_dtypes.bfloat16
    adj = np.asarray(inputs['adj'], f32)
    source = np.asarray(inputs['source'], f32)       # [B, T, N, 2]

    def rw(a):
        return (a / np.maximum(a.sum(1, keepdims=True), np.float32(1e-8))).astype(f32)

    s0t = rw(adj)            # rhs tiles: ST[n, m] with S0 = rw(adj).T
    s1t = rw(adj.T)

    def tile_nm(a):          # [N, N] -> [128, KT, N]
        return np.ascontiguousarray(a.reshape(KT, P, N).transpose(1, 0, 2))

    common = {
        's0t': tile_nm(s0t).astype(bf),
        's1t': tile_nm(s1t).astype(bf),
        'ident': np.eye(P, dtype=bf),
        'identf': np.eye(P, dtype=f32),
    }
    idhi = np.zeros((P, HID), f32)
    idhi[HID:P] = np.eye(HID, dtype=f32)
    common['idhi'] = idhi

    for L, I in (('enc0', 2), ('enc1', 64), ('dec0', 1), ('dec1', 64)):
        F = I + HID
        Wru = np.asarray(inputs[L + '_Wru'], f32)
        Wc = np.asarray(inputs[L + '_Wc'], f32)
        bru = np.asarray(inputs[L + '_bru'], f32)
        bc = np.asarray(inputs[L + '_bc'], f32)
        bA = [Wru[i * F:(i + 1) * F] for i in range(5)]
        bC = [Wc[i * F:(i + 1) * F] for i in range(5)]
        eA = [bA[0] - bA[2] - bA[4], bA[1], 2 * bA[2], bA[3], 2 * bA[4]]
        eC = [bC[0] - bC[2] - bC[4], bC[1], 2 * bC[2], bC[3], 2 * bC[4]]
        wa = np.zeros((P, 5, P), f32)
        wc = np.zeros((P, 5, HID), f32)
        for i in range(5):
            wa[0:HID, i] = eA[i][I:F]        # h-part rows 0:64
            wa[HID:HID + I, i] = eA[i][0:I]  # x-part rows 64:64+I
            if i > 0:
                wc[0:HID, i] = eC[i][I:F]
                wc[HID:HID + I, i] = eC[i][0:I]
        # gB identity: h-part via rh tile (wc0h), x-part per layer
        common[L + '_wc0h'] = np.ascontiguousarray(eC[0][I:F]).astype(bf)
        if L == 'enc0':
            wc0 = np.zeros((P, HID), f32)
            wc0[HID:HID + I] = eC[0][0:I]    # x-part read from zT0 (K=128)
            common['enc0_wc0'] = wc0
        elif L == 'dec0':
            wa0x1 = np.zeros((P, P), f32)
            wc0x1 = np.zeros((P, HID), f32)
            for cc in range(4):
                wa0x1[32 * cc] = eA[0][0]
                wc0x1[32 * cc] = eC[0][0]
            common['dec0_wa0x1'] = wa0x1.astype(bf)
            common['dec0_wc0x1'] = wc0x1.astype(bf)
            wa[HID:HID + I, 0] = 0           # y handled via K=1 terms
        else:
            common[L + '_wa0x'] = np.ascontiguousarray(eA[0][0:I])
            common[L + '_wc0x'] = np.ascontiguousarray(eC[0][0:I])
        common[L + '_wa0'] = np.ascontiguousarray(wa[:, 0, :])
        common[L + '_wa'] = wa.astype(bf)
        common[L + '_wc'] = wc.astype(bf)
        common[L + '_bru'] = bru.reshape(P, 1).copy()
        common[L + '_bc'] = bc.reshape(HID, 1).copy()

    common['zeros'] = np.zeros((P, N), f32)
    common['zerosb'] = np.zeros((P, N), bf)
    common['fcnw'] = np.asarray(inputs['fcn_W'], f32).reshape(HID, 1).copy()
    common['fcnw8'] = np.repeat(common['fcnw'], 8, axis=1).copy()
    common['fcnb'] = np.full((P, 1), np.asarray(inputs['fcn_b'], f32).reshape(-1)[0], f32)

    T = _cache['T']
    in_maps = []
    for core in range(N_CORES):
        m = dict(common)
        xT = np.zeros((CHAINS, T, 2, N), f32)
        xn = np.zeros((CHAINS, T, P, KT, 2), bf)
        for c in range(CHAINS):
            b = core * CHAINS + c
            for t in range(T):
                xt = source[b, t]            # [N, 2]
                xT[c, t] = xt.T
                xn[c, t] = xt.reshape(KT, P, 2).transpose(1, 0, 2)
        m['xT'] = xT
        m['xn'] = xn
        in_maps.append(m)
    return in_maps


def kernel(**inputs):
    from concourse import bass_utils

    T = int(os.environ.get('DCRNN_T', T_FULL))
    HOR = int(os.environ.get('DCRNN_HOR', HOR_FULL))
    key = (T, HOR)
    if _cache.get('key') != key:
        _cache['nc'] = _build(T, HOR)
        _cache['key'] = key
        _cache['T'] = T
        _cache['HOR'] = HOR
    _cache['T'] = T

    in_maps = _prep_host(inputs)
    res = bass_utils.run_bass_kernel_spmd(
        _cache['nc'], in_maps, core_ids=list(range(N_CORES)),
        trace=bool(int(os.environ.get('DCRNN_TRACE', '0'))))
    _cache['last_res'] = res

    HORr = _cache['HOR']
    out = np.zeros((HORr, B, N), np.float32)
    for core in range(N_CORES):
        r = res.results[core]['out']         # [HOR, CHAINS, N]
        for c in range(CHAINS):
            out[:, core * CHAINS + c, :] = r[:, c, :]
    return out

